# revision 3
# baseline (speedup 1.0000x reference)
"""Trainium2 Bass kernel for nn_BPFTLoss — fp8/LNS8 dual-path streaming, v4.

Quantization scheme (validated, rel err ~1.9e-5 vs the f32 reference):
  ACT share (vocab [0, V_ACT)): e4m3(x); ScalarE Exp in-place + f32
     accum_out per chunk.
  PE share  (vocab [V_ACT, V)): host log-quantizes i8 = round(8*log2e*x
     + 56 + C8) (affine only); the e4m3 bit pattern decodes to ~e^x on
     device, and TensorE ones-matmuls accumulate exact f32 row sums in
     PSUM.  C8 calibrated so the decode bias averages to ~0.
  Label logits host-gathered in f32; loss = sum w*(ln(S) - xl).

v4 schedule (from the v3 trace):
  - Full 16.4MB stream staged in SBUF (no rings, no SP gating); ~20
    back-to-back DMAs at ~350 GB/s (8-core HBM cap), chunks <= 2.1MB so
    neither engine hits completion-granularity stalls.
  - ACT chunks are per-(group, col-range) (13 ACTIVATEs; ~0.56us fixed
    cost each), group 3 tapered.  Per-group reduces run mid-stream.
  - PE tiles are [128 vocab x 512 rows] (all groups) -> 167 matmuls of
    N=512 into one PSUM [1,512]; sm copy + 4 rank-1 transpose matmuls
    overlap ACT's tapered tail.
  - Output DMA on the SP queue: the gpsimd (SWDGE) path measured ~7us
    completion latency for a 512B store; SP's HWDGE is ~1.5us.
"""

from contextlib import ExitStack

import numpy as np
import ml_dtypes

import concourse.bacc as bacc
import concourse.bass as bass
import concourse.mybir as mybir

B, S, V = 2, 2048, 32000
NCORES = 8
P = 128
G = 4
R = G * P  # 512 rows per core
V_ACT = 10624
V_PE = V - V_ACT  # 21376
T_PE = V_PE // P  # 167 vocab slices, each [128 vocab x 512 rows]
LOG2E = float(np.log2(np.e))
C8 = -0.4565451573114843

# ACT chunks: (group, width); group 3 tapered
A_PLAN = [
    (0, 5312), (0, 5312),
    (1, 5312), (1, 5312),
    (2, 5312), (2, 5312),
    (3, 5312), (3, 2656), (3, 1328), (3, 664), (3, 664),
]
for gi in range(G):
    assert sum(w for g, w in A_PLAN if g == gi) == V_ACT
NCH_A = len(A_PLAN)

# PE chunks in slices (1 slice = one [128,512] tile = 64KB)
P_PLAN = [8, 16, 32, 32, 32, 24, 12, 6, 3, 2]
assert sum(P_PLAN) == T_PE
NCH_P = len(P_PLAN)

# SP issue order (indices into A_PLAN / P_PLAN)
ISSUE = [
    ("P", 0), ("A", 0), ("P", 1), ("A", 1), ("P", 2), ("A", 2),
    ("P", 3), ("A", 3), ("P", 4), ("A", 4), ("P", 5), ("A", 5),
    ("P", 6), ("A", 6), ("P", 7), ("A", 7), ("A", 8), ("P", 8),
    ("A", 9), ("P", 9), ("A", 10),
]
assert sorted(k for t, k in ISSUE if t == "A") == list(range(NCH_A))
assert sorted(k for t, k in ISSUE if t == "P") == list(range(NCH_P))


def _plans():
    a_chunks = []  # (gi, col0, width, per-group chunk idx)
    col = {gi: 0 for gi in range(G)}
    cnt = {gi: 0 for gi in range(G)}
    for gi, wk in A_PLAN:
        a_chunks.append((gi, col[gi], wk, cnt[gi]))
        col[gi] += wk
        cnt[gi] += 1
    a_group_nch = [cnt[gi] for gi in range(G)]
    a_done_after = []  # chunks (cumulative, in A_PLAN order) when group done
    seen = {gi: 0 for gi in range(G)}
    for i, (gi, _, _, _) in enumerate(a_chunks):
        seen[gi] += 1
        if seen[gi] == a_group_nch[gi]:
            a_done_after.append((gi, i + 1))
    p_wins = []
    t = 0
    for n in P_PLAN:
        p_wins.append((t, t + n))
        t += n
    return a_chunks, a_group_nch, dict(a_done_after), p_wins


def build_kernel() -> bass.Bass:
    a_chunks, a_group_nch, a_done_after, p_wins = _plans()

    nc = bacc.Bacc("TRN2", target_bir_lowering=False, debug=False)
    xa = nc.declare_dram_parameter(
        "xa", [P, G * V_ACT], mybir.dt.float8e4, isOutput=False
    )
    xp = nc.declare_dram_parameter(
        "xp", [P, T_PE * R], mybir.dt.float8e4, isOutput=False
    )
    xl = nc.declare_dram_parameter("xl", [P, G], mybir.dt.float32, isOutput=False)
    w = nc.declare_dram_parameter("w", [P, G], mybir.dt.float32, isOutput=False)
    out = nc.declare_dram_parameter("out", [P, 1], mybir.dt.float32, isOutput=True)

    with ExitStack() as ctx:
        xa_t = ctx.enter_context(
            nc.sbuf_tensor("xa_t", [P, G * V_ACT], mybir.dt.float8e4)
        )
        xp_t = ctx.enter_context(
            nc.sbuf_tensor("xp_t", [P, T_PE * R], mybir.dt.float8e4)
        )
        ones_w = ctx.enter_context(nc.sbuf_tensor("ones_w", [P, 1], mybir.dt.float8e4))
        ones11 = ctx.enter_context(nc.sbuf_tensor("ones11", [1, 1], mybir.dt.float32))
        sums = ctx.enter_context(nc.sbuf_tensor("sums", [P, NCH_A], mybir.dt.float32))
        red = ctx.enter_context(nc.sbuf_tensor("red", [P, G], mybir.dt.float32))
        sm = ctx.enter_context(nc.sbuf_tensor("sm", [1, R], mybir.dt.float32))
        xl_t = ctx.enter_context(nc.sbuf_tensor("xl_t", [P, G], mybir.dt.float32))
        w_t = ctx.enter_context(nc.sbuf_tensor("w_t", [P, G], mybir.dt.float32))
        total = ctx.enter_context(nc.sbuf_tensor("total", [P, G], mybir.dt.float32))
        lse = ctx.enter_context(nc.sbuf_tensor("lse", [P, G], mybir.dt.float32))
        diff = ctx.enter_context(nc.sbuf_tensor("diff", [P, G], mybir.dt.float32))
        acc = ctx.enter_context(nc.sbuf_tensor("acc", [P, 1], mybir.dt.float32))
        warm = ctx.enter_context(nc.sbuf_tensor("warm", [P, 512], mybir.dt.float8e4))
        pm = ctx.enter_context(nc.psum_tensor("pm", [1, R], mybir.dt.float32))
        p2 = ctx.enter_context(nc.psum_tensor("p2", [P, G], mybir.dt.float32))

        s_a = [ctx.enter_context(nc.semaphore(f"s_ac{k}")) for k in range(NCH_A)]
        s_p = [ctx.enter_context(nc.semaphore(f"s_pc{k}")) for k in range(NCH_P)]
        s_xl = ctx.enter_context(nc.semaphore("s_xl"))
        s_w = ctx.enter_context(nc.semaphore("s_w"))
        s_ones = ctx.enter_context(nc.semaphore("s_ones"))
        s_act = ctx.enter_context(nc.semaphore("s_act"))
        s_mm = ctx.enter_context(nc.semaphore("s_mm"))
        s_sm = ctx.enter_context(nc.semaphore("s_sm"))
        s_p2 = ctx.enter_context(nc.semaphore("s_p2"))
        s_red = ctx.enter_context(nc.semaphore("s_red"))
        s_tot = ctx.enter_context(nc.semaphore("s_tot"))
        s_ln = ctx.enter_context(nc.semaphore("s_ln"))
        s_fin = ctx.enter_context(nc.semaphore("s_fin"))
        s_out = ctx.enter_context(nc.semaphore("s_out"))
        s_warm = ctx.enter_context(nc.semaphore("s_warm"))

        block = ctx.enter_context(nc.Block())

        @block.sync
        def _(sync: bass.BassEngine):
            for kind, k in ISSUE:
                if kind == "A":
                    gi, col0, wk, _ = a_chunks[k]
                    o = gi * V_ACT + col0
                    sync.dma_start(
                        out=xa_t[:, o : o + wk], in_=xa[:, o : o + wk]
                    ).then_inc(s_a[k], 16)
                else:
                    t0, t1 = p_wins[k]
                    sync.dma_start(
                        out=xp_t[:, t0 * R : t1 * R], in_=xp[:, t0 * R : t1 * R]
                    ).then_inc(s_p[k], 16)
            # warmer DMAs: keep the DGE/completion path hot through the
            # tail so the final 512B store doesn't pay a ~7us cold-start
            sync.wait_ge(s_mm, NCH_P)
            sync.dma_start(out=warm[:], in_=xa[:, 0:512]).then_inc(s_warm, 16)
            sync.wait_ge(s_red, G)
            sync.dma_start(out=warm[:], in_=xa[:, 512:1024]).then_inc(s_warm, 16)
            sync.wait_ge(s_fin, 3)
            sync.dma_start(out=out[:], in_=acc[:]).then_inc(s_out, 16)
            sync.wait_ge(s_out, 16)

        @block.scalar
        def _(scalar: bass.BassEngine):
            for k, (gi, col0, wk, _) in enumerate(a_chunks):
                o = gi * V_ACT + col0
                scalar.wait_ge(s_a[k], 16)
                scalar.activation(
                    out=xa_t[:, o : o + wk],
                    in_=xa_t[:, o : o + wk],
                    func=mybir.ActivationFunctionType.Exp,
                    accum_out=sums[:, k : k + 1],
                ).then_inc(s_act, 1)
            scalar.wait_ge(s_tot, 1)
            scalar.activation(
                out=lse[:], in_=total[:], func=mybir.ActivationFunctionType.Ln
            ).then_inc(s_ln, 1)

        @block.tensor
        def _(tensor: bass.BassEngine):
            tensor.wait_ge(s_ones, 1)
            for k, (t0, t1) in enumerate(p_wins):
                tensor.wait_ge(s_p[k], 16)
                for t in range(t0, t1):
                    ins = tensor.matmul(
                        out=pm[:],
                        lhsT=ones_w[:],
                        rhs=xp_t[:, t * R : (t + 1) * R],
                        start=(t == 0),
                        stop=(t == T_PE - 1),
                    )
                    if t == t1 - 1:
                        ins.then_inc(s_mm, 1)
            tensor.wait_ge(s_sm, 1)
            for gi in range(G):
                tensor.matmul(
                    out=p2[:, gi : gi + 1],
                    lhsT=sm[:, gi * P : (gi + 1) * P],
                    rhs=ones11[:],
                    start=True,
                    stop=True,
                ).then_inc(s_p2, 1)

        @block.vector
        def _(vector: bass.BassEngine):
            vector.memset(ones_w[:], 1.0)
            vector.memset(ones11[:], 1.0).then_inc(s_ones, 1)
            # per-group exp-sum reduces as each group's ACT chunks finish
            for gi in range(G):
                vector.wait_ge(s_act, a_done_after[gi])
                cols = [
                    k for k, (g, _, _, _) in enumerate(a_chunks) if g == gi
                ]
                c0, c1 = min(cols), max(cols) + 1
                assert cols == list(range(c0, c1))
                vector.reduce_sum(
                    out=red[:, gi : gi + 1],
                    in_=sums[:, c0:c1],
                    axis=mybir.AxisListType.X,
                ).then_inc(s_red, 1)
            vector.wait_ge(s_mm, NCH_P)
            vector.tensor_copy(out=sm[:], in_=pm[:]).then_inc(s_sm, 1)
            # DVE pipelines (depth 8): serialize same-engine RAW chains
            vector.wait_ge(s_p2, G)
            vector.wait_ge(s_red, G)
            vector.tensor_add(out=total[:], in0=red[:], in1=p2[:]).then_inc(s_tot, 1)
            vector.wait_ge(s_ln, 1)
            vector.wait_ge(s_xl, 16)
            vector.tensor_sub(out=diff[:], in0=lse[:], in1=xl_t[:]).then_inc(s_fin, 1)
            vector.wait_ge(s_w, 16)
            vector.wait_ge(s_fin, 1)
            vector.tensor_mul(out=diff[:], in0=diff[:], in1=w_t[:]).then_inc(s_fin, 1)
            vector.wait_ge(s_fin, 2)
            vector.reduce_sum(
                out=acc[:], in_=diff[:], axis=mybir.AxisListType.X
            ).then_inc(s_fin, 1)

        @block.gpsimd
        def _(gpsimd: bass.BassEngine):
            gpsimd.dma_start(out=xl_t[:], in_=xl[:]).then_inc(s_xl, 16)
            gpsimd.dma_start(out=w_t[:], in_=w[:]).then_inc(s_w, 16)
            gpsimd.wait_ge(s_out, 16)

    orig_tables = bacc.get_activation_tables

    def _patched_tables(arch):
        t = orig_tables(arch)
        for k in ("exp_and_others", "exp_and_friends", "natural_log"):
            if k in t:
                t[k] = set()
        return t

    bacc.get_activation_tables = _patched_tables
    try:
        nc.finalize()
    finally:
        bacc.get_activation_tables = orig_tables
    return nc


_BUILT: list = []


def _get_built() -> bass.Bass:
    if not _BUILT:
        _BUILT.append(build_kernel())
    return _BUILT[0]


def prepare_in_maps(logits, labels, factuality_scores):
    logits = np.asarray(logits)
    labels = np.asarray(labels)
    fs = np.asarray(factuality_scores, dtype=np.float64)
    assert logits.shape == (B, S, V), logits.shape

    rpc = (B * S) // NCORES
    x2d = logits.reshape(B * S, V)

    n_loss_rows = B * (S - 1)
    lab_next = np.zeros((B, S), np.int64)
    lab_next[:, :-1] = labels[:, 1:]
    lab_flat = lab_next.reshape(-1)
    wmat = np.zeros((B, S), np.float64)
    wmat[:, :-1] = ((2.0 - fs) / n_loss_rows)[:, None]
    w_flat = wmat.reshape(-1).astype(np.float32)
    xl_flat = x2d[np.arange(B * S), lab_flat]

    xa8 = x2d[:, :V_ACT].astype(ml_dtypes.float8_e4m3fn)
    i8 = np.clip(
        np.rint(x2d[:, V_ACT:] * np.float32(8 * LOG2E) + np.float32(56 + C8)),
        0,
        126,
    ).astype(np.uint8)

    in_maps = []
    for c in range(NCORES):
        sl = slice(c * rpc, (c + 1) * rpc)
        # xa[p, gi*V_ACT + j] = e4m3(x[gi*128 + p, j])
        xa_c = np.ascontiguousarray(
            xa8[sl].reshape(G, P, V_ACT).transpose(1, 0, 2).reshape(P, G * V_ACT)
        )
        # xp[p, t*512 + r] = lns8(x[row r, V_ACT + t*128 + p])
        xp_c = np.ascontiguousarray(
            i8[sl].reshape(R, T_PE, P).transpose(2, 1, 0).reshape(P, T_PE * R)
        ).view(ml_dtypes.float8_e4m3fn)
        xl_c = np.ascontiguousarray(xl_flat[sl].reshape(G, P).T)
        w_c = np.ascontiguousarray(w_flat[sl].reshape(G, P).T)
        in_maps.append({"xa": xa_c, "xp": xp_c, "xl": xl_c, "w": w_c})
    return in_maps


def kernel(logits, labels, factuality_scores, contradiction_scores):
    from concourse.bass_utils import run_bass_kernel_spmd

    nc = _get_built()
    in_maps = prepare_in_maps(logits, labels, factuality_scores)
    res = run_bass_kernel_spmd(nc, in_maps, list(range(NCORES)))
    total = 0.0
    for r in res.results:
        total += r["out"].astype(np.float64).sum()
    return np.asarray(total, dtype=np.float32)


# revision 4
# speedup vs baseline: 1.1964x; 1.1964x over previous
"""Trainium2 Bass kernel for nn_BPFTLoss — fp8/LNS8 dual-path streaming, v4.

Quantization scheme (validated, rel err ~1.9e-5 vs the f32 reference):
  ACT share (vocab [0, V_ACT)): e4m3(x); ScalarE Exp in-place + f32
     accum_out per chunk.
  PE share  (vocab [V_ACT, V)): host log-quantizes i8 = round(8*log2e*x
     + 56 + C8) (affine only); the e4m3 bit pattern decodes to ~e^x on
     device, and TensorE ones-matmuls accumulate exact f32 row sums in
     PSUM.  C8 calibrated so the decode bias averages to ~0.
  Label logits host-gathered in f32; loss = sum w*(ln(S) - xl).

v4 schedule (from the v3 trace):
  - Full 16.4MB stream staged in SBUF (no rings, no SP gating); ~20
    back-to-back DMAs at ~350 GB/s (8-core HBM cap), chunks <= 2.1MB so
    neither engine hits completion-granularity stalls.
  - ACT chunks are per-(group, col-range) (13 ACTIVATEs; ~0.56us fixed
    cost each), group 3 tapered.  Per-group reduces run mid-stream.
  - PE tiles are [128 vocab x 512 rows] (all groups) -> 167 matmuls of
    N=512 into one PSUM [1,512]; sm copy + 4 rank-1 transpose matmuls
    overlap ACT's tapered tail.
  - Output DMA on the SP queue: the gpsimd (SWDGE) path measured ~7us
    completion latency for a 512B store; SP's HWDGE is ~1.5us.
"""

from contextlib import ExitStack

import numpy as np
import ml_dtypes

import concourse.bacc as bacc
import concourse.bass as bass
import concourse.mybir as mybir

B, S, V = 2, 2048, 32000
NCORES = 8
P = 128
G = 4
R = G * P  # 512 rows per core
V_ACT = 10624
V_PE = V - V_ACT  # 21376
T_PE = V_PE // P  # 167 vocab slices, each [128 vocab x 512 rows]
LOG2E = float(np.log2(np.e))
C8 = -0.4565451573114843

# ACT chunks: (group, width); group 3 tapered
A_PLAN = [
    (0, 5312), (0, 5312),
    (1, 5312), (1, 5312),
    (2, 5312), (2, 5312),
    (3, 5312), (3, 2656), (3, 1328), (3, 664), (3, 664),
]
for gi in range(G):
    assert sum(w for g, w in A_PLAN if g == gi) == V_ACT
NCH_A = len(A_PLAN)

# PE chunks in slices (1 slice = one [128,512] tile = 64KB)
P_PLAN = [8, 16, 32, 32, 32, 24, 12, 6, 3, 2]
assert sum(P_PLAN) == T_PE
NCH_P = len(P_PLAN)

# SP issue order (indices into A_PLAN / P_PLAN)
ISSUE = [
    ("P", 0), ("A", 0), ("P", 1), ("A", 1), ("P", 2), ("A", 2),
    ("P", 3), ("A", 3), ("P", 4), ("A", 4), ("P", 5), ("A", 5),
    ("P", 6), ("A", 6), ("P", 7), ("A", 7), ("A", 8), ("P", 8),
    ("A", 9), ("P", 9), ("A", 10),
]
assert sorted(k for t, k in ISSUE if t == "A") == list(range(NCH_A))
assert sorted(k for t, k in ISSUE if t == "P") == list(range(NCH_P))


def _plans():
    a_chunks = []  # (gi, col0, width, per-group chunk idx)
    col = {gi: 0 for gi in range(G)}
    cnt = {gi: 0 for gi in range(G)}
    for gi, wk in A_PLAN:
        a_chunks.append((gi, col[gi], wk, cnt[gi]))
        col[gi] += wk
        cnt[gi] += 1
    a_group_nch = [cnt[gi] for gi in range(G)]
    a_done_after = []  # chunks (cumulative, in A_PLAN order) when group done
    seen = {gi: 0 for gi in range(G)}
    for i, (gi, _, _, _) in enumerate(a_chunks):
        seen[gi] += 1
        if seen[gi] == a_group_nch[gi]:
            a_done_after.append((gi, i + 1))
    p_wins = []
    t = 0
    for n in P_PLAN:
        p_wins.append((t, t + n))
        t += n
    return a_chunks, a_group_nch, dict(a_done_after), p_wins


def build_kernel() -> bass.Bass:
    a_chunks, a_group_nch, a_done_after, p_wins = _plans()

    nc = bacc.Bacc("TRN2", target_bir_lowering=False, debug=False)
    xa = nc.declare_dram_parameter(
        "xa", [P, G * V_ACT], mybir.dt.float8e4, isOutput=False
    )
    xp = nc.declare_dram_parameter(
        "xp", [P, T_PE * R], mybir.dt.float8e4, isOutput=False
    )
    xl = nc.declare_dram_parameter("xl", [P, G], mybir.dt.float32, isOutput=False)
    w = nc.declare_dram_parameter("w", [P, G], mybir.dt.float32, isOutput=False)
    out = nc.declare_dram_parameter("out", [P, 1], mybir.dt.float32, isOutput=True)

    with ExitStack() as ctx:
        xa_t = ctx.enter_context(
            nc.sbuf_tensor("xa_t", [P, G * V_ACT], mybir.dt.float8e4)
        )
        xp_t = ctx.enter_context(
            nc.sbuf_tensor("xp_t", [P, T_PE * R], mybir.dt.float8e4)
        )
        ones_w = ctx.enter_context(nc.sbuf_tensor("ones_w", [P, 1], mybir.dt.float8e4))
        ones11 = ctx.enter_context(nc.sbuf_tensor("ones11", [1, 1], mybir.dt.float32))
        sums = ctx.enter_context(nc.sbuf_tensor("sums", [P, NCH_A], mybir.dt.float32))
        red = ctx.enter_context(nc.sbuf_tensor("red", [P, G], mybir.dt.float32))
        sm = ctx.enter_context(nc.sbuf_tensor("sm", [1, R], mybir.dt.float32))
        xl_t = ctx.enter_context(nc.sbuf_tensor("xl_t", [P, G], mybir.dt.float32))
        w_t = ctx.enter_context(nc.sbuf_tensor("w_t", [P, G], mybir.dt.float32))
        total = ctx.enter_context(nc.sbuf_tensor("total", [P, G], mybir.dt.float32))
        lse = ctx.enter_context(nc.sbuf_tensor("lse", [P, G], mybir.dt.float32))
        diff = ctx.enter_context(nc.sbuf_tensor("diff", [P, G], mybir.dt.float32))
        acc = ctx.enter_context(nc.sbuf_tensor("acc", [P, 1], mybir.dt.float32))
        pm = ctx.enter_context(nc.psum_tensor("pm", [1, R], mybir.dt.float32))
        p2 = ctx.enter_context(nc.psum_tensor("p2", [P, G], mybir.dt.float32))

        s_a = [ctx.enter_context(nc.semaphore(f"s_ac{k}")) for k in range(NCH_A)]
        s_p = [ctx.enter_context(nc.semaphore(f"s_pc{k}")) for k in range(NCH_P)]
        s_xl = ctx.enter_context(nc.semaphore("s_xl"))
        s_w = ctx.enter_context(nc.semaphore("s_w"))
        s_ones = ctx.enter_context(nc.semaphore("s_ones"))
        s_act = ctx.enter_context(nc.semaphore("s_act"))
        s_mm = ctx.enter_context(nc.semaphore("s_mm"))
        s_sm = ctx.enter_context(nc.semaphore("s_sm"))
        s_p2 = ctx.enter_context(nc.semaphore("s_p2"))
        s_red = ctx.enter_context(nc.semaphore("s_red"))
        s_tot = ctx.enter_context(nc.semaphore("s_tot"))
        s_ln = ctx.enter_context(nc.semaphore("s_ln"))
        s_fin = ctx.enter_context(nc.semaphore("s_fin"))
        s_out = ctx.enter_context(nc.semaphore("s_out"))

        block = ctx.enter_context(nc.Block())

        @block.sync
        def _(sync: bass.BassEngine):
            for kind, k in ISSUE:
                if kind == "A":
                    gi, col0, wk, _ = a_chunks[k]
                    o = gi * V_ACT + col0
                    sync.dma_start(
                        out=xa_t[:, o : o + wk], in_=xa[:, o : o + wk]
                    ).then_inc(s_a[k], 16)
                else:
                    t0, t1 = p_wins[k]
                    sync.dma_start(
                        out=xp_t[:, t0 * R : t1 * R], in_=xp[:, t0 * R : t1 * R]
                    ).then_inc(s_p[k], 16)
            sync.wait_ge(s_fin, 3)
            sync.dma_start(out=out[:], in_=acc[:]).then_inc(s_out, 16)
            sync.wait_ge(s_out, 16)

        @block.scalar
        def _(scalar: bass.BassEngine):
            for k, (gi, col0, wk, _) in enumerate(a_chunks):
                o = gi * V_ACT + col0
                scalar.wait_ge(s_a[k], 16)
                scalar.activation(
                    out=xa_t[:, o : o + wk],
                    in_=xa_t[:, o : o + wk],
                    func=mybir.ActivationFunctionType.Exp,
                    accum_out=sums[:, k : k + 1],
                ).then_inc(s_act, 1)
            scalar.wait_ge(s_tot, 1)
            scalar.activation(
                out=lse[:], in_=total[:], func=mybir.ActivationFunctionType.Ln
            ).then_inc(s_ln, 1)

        @block.tensor
        def _(tensor: bass.BassEngine):
            tensor.wait_ge(s_ones, 1)
            for k, (t0, t1) in enumerate(p_wins):
                tensor.wait_ge(s_p[k], 16)
                for t in range(t0, t1):
                    ins = tensor.matmul(
                        out=pm[:],
                        lhsT=ones_w[:],
                        rhs=xp_t[:, t * R : (t + 1) * R],
                        start=(t == 0),
                        stop=(t == T_PE - 1),
                    )
                    if t == t1 - 1:
                        ins.then_inc(s_mm, 1)
            tensor.wait_ge(s_sm, 1)
            for gi in range(G):
                tensor.matmul(
                    out=p2[:, gi : gi + 1],
                    lhsT=sm[:, gi * P : (gi + 1) * P],
                    rhs=ones11[:],
                    start=True,
                    stop=True,
                ).then_inc(s_p2, 1)

        @block.vector
        def _(vector: bass.BassEngine):
            vector.memset(ones_w[:], 1.0)
            vector.memset(ones11[:], 1.0).then_inc(s_ones, 1)
            # per-group exp-sum reduces as each group's ACT chunks finish
            for gi in range(G):
                vector.wait_ge(s_act, a_done_after[gi])
                cols = [
                    k for k, (g, _, _, _) in enumerate(a_chunks) if g == gi
                ]
                c0, c1 = min(cols), max(cols) + 1
                assert cols == list(range(c0, c1))
                vector.reduce_sum(
                    out=red[:, gi : gi + 1],
                    in_=sums[:, c0:c1],
                    axis=mybir.AxisListType.X,
                ).then_inc(s_red, 1)
            vector.wait_ge(s_mm, NCH_P)
            vector.tensor_copy(out=sm[:], in_=pm[:]).then_inc(s_sm, 1)
            # DVE pipelines (depth 8): serialize same-engine RAW chains
            vector.wait_ge(s_p2, G)
            vector.wait_ge(s_red, G)
            vector.tensor_add(out=total[:], in0=red[:], in1=p2[:]).then_inc(s_tot, 1)
            vector.wait_ge(s_ln, 1)
            vector.wait_ge(s_xl, 16)
            vector.tensor_sub(out=diff[:], in0=lse[:], in1=xl_t[:]).then_inc(s_fin, 1)
            vector.wait_ge(s_w, 16)
            vector.wait_ge(s_fin, 1)
            vector.tensor_mul(out=diff[:], in0=diff[:], in1=w_t[:]).then_inc(s_fin, 1)
            vector.wait_ge(s_fin, 2)
            vector.reduce_sum(
                out=acc[:], in_=diff[:], axis=mybir.AxisListType.X
            ).then_inc(s_fin, 1)

        @block.gpsimd
        def _(gpsimd: bass.BassEngine):
            gpsimd.dma_start(out=xl_t[:], in_=xl[:]).then_inc(s_xl, 16)
            gpsimd.dma_start(out=w_t[:], in_=w[:]).then_inc(s_w, 16)
            gpsimd.wait_ge(s_out, 16)

    orig_tables = bacc.get_activation_tables

    def _patched_tables(arch):
        t = orig_tables(arch)
        for k in ("exp_and_others", "exp_and_friends", "natural_log"):
            if k in t:
                t[k] = set()
        return t

    bacc.get_activation_tables = _patched_tables
    try:
        nc.finalize()
    finally:
        bacc.get_activation_tables = orig_tables
    return nc


_BUILT: list = []


def _get_built() -> bass.Bass:
    if not _BUILT:
        _BUILT.append(build_kernel())
    return _BUILT[0]


def prepare_in_maps(logits, labels, factuality_scores):
    logits = np.asarray(logits)
    labels = np.asarray(labels)
    fs = np.asarray(factuality_scores, dtype=np.float64)
    assert logits.shape == (B, S, V), logits.shape

    rpc = (B * S) // NCORES
    x2d = logits.reshape(B * S, V)

    n_loss_rows = B * (S - 1)
    lab_next = np.zeros((B, S), np.int64)
    lab_next[:, :-1] = labels[:, 1:]
    lab_flat = lab_next.reshape(-1)
    wmat = np.zeros((B, S), np.float64)
    wmat[:, :-1] = ((2.0 - fs) / n_loss_rows)[:, None]
    w_flat = wmat.reshape(-1).astype(np.float32)
    xl_flat = x2d[np.arange(B * S), lab_flat]

    xa8 = x2d[:, :V_ACT].astype(ml_dtypes.float8_e4m3fn)
    i8 = np.clip(
        np.rint(x2d[:, V_ACT:] * np.float32(8 * LOG2E) + np.float32(56 + C8)),
        0,
        126,
    ).astype(np.uint8)

    in_maps = []
    for c in range(NCORES):
        sl = slice(c * rpc, (c + 1) * rpc)
        # xa[p, gi*V_ACT + j] = e4m3(x[gi*128 + p, j])
        xa_c = np.ascontiguousarray(
            xa8[sl].reshape(G, P, V_ACT).transpose(1, 0, 2).reshape(P, G * V_ACT)
        )
        # xp[p, t*512 + r] = lns8(x[row r, V_ACT + t*128 + p])
        xp_c = np.ascontiguousarray(
            i8[sl].reshape(R, T_PE, P).transpose(2, 1, 0).reshape(P, T_PE * R)
        ).view(ml_dtypes.float8_e4m3fn)
        xl_c = np.ascontiguousarray(xl_flat[sl].reshape(G, P).T)
        w_c = np.ascontiguousarray(w_flat[sl].reshape(G, P).T)
        in_maps.append({"xa": xa_c, "xp": xp_c, "xl": xl_c, "w": w_c})
    return in_maps


def kernel(logits, labels, factuality_scores, contradiction_scores):
    from concourse.bass_utils import run_bass_kernel_spmd

    nc = _get_built()
    in_maps = prepare_in_maps(logits, labels, factuality_scores)
    res = run_bass_kernel_spmd(nc, in_maps, list(range(NCORES)))
    total = 0.0
    for r in res.results:
        total += r["out"].astype(np.float64).sum()
    return np.asarray(total, dtype=np.float32)
